# revision 3
# baseline (speedup 1.0000x reference)
"""Segment-mean (GNN mean-encoder) Trainium2 kernel.

Strategy (per the node-sharding variant of the sharding hint):
  * Host: partition nodes across the 8 cores round-robin in degree-sorted
    order, and repack the edge features into a jagged-diagonal (JDS) layout:
    slot j holds the j-th edge of every node that has > j edges.  Nodes are
    ranked by in-degree (descending), so slot j covers a contiguous prefix
    of ranks and the whole per-core tensor becomes one dense
    [128, SumB*D] array (rank r -> partition r%128, block r//128),
    padded only up to 128-row slot boundaries (~1.5% overhead).
    The repacked stream is stored in float16: the segment-mean tolerates
    half precision easily (l2 rel err ~1e-3 vs the 2e-2 gate), and it
    halves HBM traffic (the bottleneck) while doubling DVE add throughput
    (2x perf mode needs all operands 2-byte).
  * Device (one SPMD program on 8 NeuronCores): stream the dense array in
    column tiles, accumulate each slot's segment into a persistent
    [128, B*D] f16 accumulator with DVE adds (slot 0 is a straight copy,
    so no memset of the accumulator is needed), compute 1/max(count,1)
    from the host-packed per-rank degree vector, multiply as blocks
    finalize, and DMA the f16 result out (host casts back to f32).
  * Host: inverse-permute the per-core outputs back to node order.

No cross-core communication is needed: each core owns a disjoint node set.
"""

import numpy as np
import ml_dtypes

import concourse.bass as bass
import concourse.tile as tile
from concourse import mybir
from concourse.bass_utils import run_bass_kernel_spmd

P = 128          # SBUF partitions
NCORES = 8
D = 32           # feature dim
N = 100000       # nodes
E = 1600000      # edges
CHUNK_BLOCKS = 256   # D-element column blocks per streamed DMA tile
STREAM_BUFS = 6      # in-flight stream tiles

# test-harness hooks (the grading harness just calls kernel())
TRACE = False
TRACE_KWARGS = {}
LAST_RESULT = None


def _preprocess(e, dst):
    """Build per-core JDS arrays (f16) + per-rank counts and the inverse
    permutation."""
    counts = np.bincount(dst, minlength=N)
    maxdeg = int(counts.max())
    order = np.argsort(-counts, kind="stable")          # nodes, degree desc
    inv = np.empty(N, np.int64)
    inv[order] = np.arange(N)
    core_of = inv % NCORES
    rank_of = inv // NCORES
    m = N // NCORES                                      # nodes per core
    B = (m + P - 1) // P                                 # accumulator blocks

    counts_sorted = counts[order]
    L = np.zeros((NCORES, maxdeg), np.int64)             # slot lengths
    for c in range(NCORES):
        cc = counts_sorted[c::NCORES]
        hist = np.bincount(cc, minlength=maxdeg + 1)
        L[c, :] = m - np.cumsum(hist)[:maxdeg]
    Bj = np.max((L + P - 1) // P, axis=0)                # blocks per slot
    Cj = np.concatenate([[0], np.cumsum(Bj)]).astype(np.int64)
    SumB = int(Cj[-1])

    # per-edge slot index = occurrence index within its dst group
    perm = np.argsort(dst, kind="stable")
    sd = dst[perm]
    newgrp = np.r_[True, sd[1:] != sd[:-1]]
    starts = np.flatnonzero(newgrp)
    group_id = np.cumsum(newgrp.astype(np.int64)) - 1
    j_e = np.arange(E, dtype=np.int64) - starts[group_id]

    c_e = core_of[sd]
    r_e = rank_of[sd]
    flat_idx = (r_e % P) * SumB + Cj[j_e] + (r_e // P)   # row in [P*SumB, D]

    e_jds = np.zeros((NCORES, P * SumB, D), np.float16)
    for c in range(NCORES):
        mask = c_e == c
        e_jds[c, flat_idx[mask]] = e[perm[mask]].astype(np.float16)

    # per-rank in-degree, packed rank r -> [r % P, r // P]; exact in f16
    # (counts <= maxdeg << 2048).  Ranks >= m (padding) get 0 -> output 0.
    cnt = np.zeros((NCORES, P * B), np.float16)
    for c in range(NCORES):
        cnt[c, :m] = counts_sorted[c::NCORES]
    cnt_pb = np.ascontiguousarray(
        cnt.reshape(NCORES, B, P).transpose(0, 2, 1)     # [c, P, B]
    )

    return e_jds, cnt_pb, order, Bj, Cj, SumB, maxdeg, B, m


def _split_multi_waits(nc):
    """Walrus in this toolchain rejects instructions with more than one sem
    wait ("Too many sync wait commands").  Tile's wait assignment is not
    transitively minimal, so e.g. a DMA reusing a pool slot waits on both the
    consumer engine's sem and its own lane's previous DMA.  Hoist all but one
    wait of each instruction onto same-engine NoOps inserted right before it:
    the sequencer executes them in order, so semantics are identical.
    """
    ctr = 0
    for fn in nc.m.functions:
        for bb in fn.blocks:
            new_insts = []
            for inst in bb.instructions:
                si = inst.sync_info
                if si is not None and si.on_wait and len(si.on_wait) > 1:
                    waits = list(si.on_wait)
                    for w in waits[:-1]:
                        ctr += 1
                        nop = mybir.InstNoOp(
                            name=f"I-waitsplit-{ctr}",
                            engine=inst.engine,
                            ins=[],
                            outs=[],
                            sync_info=mybir.SyncInfo(on_wait=[w], on_update=[]),
                        )
                        new_insts.append(nop)
                    si.on_wait = [waits[-1]]
                new_insts.append(inst)
            bb.instructions = new_insts


def _chunk_bounds(SumB, chunk_blocks, taper):
    """Column-tile boundaries: fixed-size chunks, tapering down at the end of
    the stream so the final DMA->add->mul->store dependency chain is short."""
    bounds = [0]
    tail = sum(taper)
    body_end = max(0, SumB - tail)
    while bounds[-1] < body_end:
        nxt = min(bounds[-1] + chunk_blocks, body_end)
        # avoid a tiny straggler right before the taper
        if body_end - nxt < chunk_blocks // 2:
            nxt = body_end
        bounds.append(nxt)
    for tp in taper:
        if bounds[-1] < SumB:
            bounds.append(min(SumB, bounds[-1] + tp))
    while bounds[-1] < SumB:
        bounds.append(SumB)
    return bounds


def _build_program(
    SumB,
    Bj,
    Cj,
    maxdeg,
    B,
    repeats=1,
    loop_repeats=None,
    chunk_blocks=None,
    stream_bufs=None,
    taper=(64, 32, 16, 8),
    min_fin_blocks=8,
    store_engine="scalar",
):
    chunk_blocks = chunk_blocks or CHUNK_BLOCKS
    stream_bufs = stream_bufs or STREAM_BUFS
    nc = bass.Bass()
    f16 = mybir.dt.float16
    ejds = nc.dram_tensor("ejds", [P, SumB * D], f16, kind="ExternalInput")
    cnts = nc.dram_tensor("cnts", [P, B], f16, kind="ExternalInput")
    out = nc.dram_tensor("out", [P, B * D], f16, kind="ExternalOutput")

    bounds = _chunk_bounds(SumB, chunk_blocks, taper)
    Bj_l = [int(x) for x in Bj] + [0]
    Cj_l = [int(x) for x in Cj]
    store_eng = getattr(nc, store_engine)

    with tile.TileContext(nc) as tc:
        with (
            tc.tile_pool(name="acc", bufs=1) as acc_pool,
            tc.tile_pool(name="small", bufs=2) as small_pool,
            tc.tile_pool(name="stream", bufs=stream_bufs) as stream_pool,
        ):
            A = acc_pool.tile([P, B * D], f16)

            def emit_body():
                # recip = 1/max(count,1); runs in the DVE's early-stream
                # idle window
                cnt_sb = small_pool.tile([P, B], f16, tag="cnt_sb")
                nc.sync.dma_start(cnt_sb[:], cnts[:])
                recip = small_pool.tile([P, B], f16, tag="recip")
                nc.vector.tensor_scalar_max(recip[:], cnt_sb[:], 1.0)
                with nc.allow_low_precision(
                    reason="f16 mean is well within the 2e-2 error gate"
                ):
                    nc.vector.reciprocal(recip[:], recip[:])

                # slot 0 covers blocks [0, Bj[0]) and is a straight copy
                # (4x DVE mode); any blocks no slot touches must be zero
                if Bj_l[0] < B:
                    nc.vector.memset(A[:, Bj_l[0] * D:], 0.0)

                # finalized := blocks >= fin_lo are multiplied + stored
                fin_lo = [B]

                # measured on HW: finalize multiplies must stay on DVE --
                # GpSimd's SBUF port is shared with DVE, so routing them to
                # POOL slows the whole stream down (78us vs 68us)
                def finalize_down_to(b0):
                    b1 = fin_lo[0]
                    if b1 <= b0:
                        return
                    nc.vector.tensor_mul(
                        A[:, b0 * D: b1 * D].rearrange(
                            "p (b d) -> p b d", d=D
                        ),
                        A[:, b0 * D: b1 * D].rearrange(
                            "p (b d) -> p b d", d=D
                        ),
                        recip[:, b0:b1, None].broadcast_to([P, b1 - b0, D]),
                    )
                    store_eng.dma_start(
                        out[:, b0 * D: b1 * D], A[:, b0 * D: b1 * D]
                    )
                    fin_lo[0] = b0

                # stream the JDS array; each slot-aligned segment adds into A.
                # When slot j's columns end, blocks [Bj[j+1], Bj[j]) are final
                # (later slots only touch lower blocks): multiply by recip and
                # store them, merged into >= min_fin_blocks batches.
                for t in range(len(bounds) - 1):
                    blk0, blk1 = bounds[t], bounds[t + 1]
                    w = blk1 - blk0
                    tl = stream_pool.tile(
                        [P, chunk_blocks * D], f16, tag="stream"
                    )
                    nc.sync.dma_start(
                        tl[:, : w * D], ejds[:, blk0 * D: blk1 * D]
                    )
                    j = int(np.searchsorted(Cj, blk0, side="right")) - 1
                    while j < maxdeg and Cj_l[j] < blk1:
                        s0 = max(blk0, Cj_l[j])
                        s1 = min(blk1, Cj_l[j + 1])
                        if s1 > s0:
                            alo = (s0 - Cj_l[j]) * D
                            if j == 0:
                                nc.vector.tensor_copy(
                                    A[:, alo: alo + (s1 - s0) * D],
                                    tl[:, (s0 - blk0) * D: (s1 - blk0) * D],
                                )
                            else:
                                nc.vector.tensor_add(
                                    A[:, alo: alo + (s1 - s0) * D],
                                    A[:, alo: alo + (s1 - s0) * D],
                                    tl[:, (s0 - blk0) * D: (s1 - blk0) * D],
                                )
                            if s1 == Cj_l[j + 1] and (
                                fin_lo[0] - Bj_l[j + 1] >= min_fin_blocks
                                or j == maxdeg - 1
                            ):
                                finalize_down_to(Bj_l[j + 1])
                        j += 1
                finalize_down_to(0)

            if loop_repeats is not None:
                with tc.For_i(0, loop_repeats, 1):
                    emit_body()
            else:
                for _rep in range(repeats):
                    emit_body()
    _split_multi_waits(nc)
    return nc


def _make_in_maps(e_jds, cnt_pb, SumB):
    return [
        {"ejds": e_jds[c].reshape(P, SumB * D), "cnts": cnt_pb[c]}
        for c in range(NCORES)
    ]


def _make_runner(nc, in_maps):
    """Build a repeat-callable PJRT runner with inputs staged on-device once.

    Mirrors bass2jax.run_bass_via_pjrt's multi-core path, minus output-buffer
    donation (so the staged arrays can be reused across timing calls).
    """
    import jax
    from jax.experimental.shard_map import shard_map
    from jax.sharding import Mesh, NamedSharding, PartitionSpec

    from concourse import bass2jax

    bass2jax.install_neuronx_cc_hook()
    n_cores = len(in_maps)

    partition_name = (
        nc.partition_id_tensor.name if nc.partition_id_tensor else None
    )
    in_names, out_names, out_avals, zero_outs = [], [], [], []
    for alloc in nc.m.functions[0].allocations:
        if not isinstance(alloc, mybir.MemoryLocationSet):
            continue
        name = alloc.memorylocations[0].name
        if alloc.kind == "ExternalInput":
            if name != partition_name:
                in_names.append(name)
        elif alloc.kind == "ExternalOutput":
            out_names.append(name)
            shape = tuple(alloc.tensor_shape)
            dtype = mybir.dt.np(alloc.dtype)
            out_avals.append(jax.core.ShapedArray(shape, dtype))
            zero_outs.append(np.zeros(shape, dtype))
    n_params = len(in_names)
    all_names = in_names + out_names
    if partition_name is not None:
        all_names = all_names + [partition_name]

    def _body(*args):
        operands = list(args)
        if partition_name is not None:
            operands.append(bass2jax.partition_id_tensor())
        outs = bass2jax._bass_exec_p.bind(
            *operands,
            out_avals=tuple(out_avals),
            in_names=tuple(all_names),
            out_names=tuple(out_names),
            lowering_input_output_aliases=(),
            sim_require_finite=True,
            sim_require_nnan=True,
            nc=nc,
        )
        return tuple(outs)

    devices = jax.devices()[:n_cores]
    mesh = Mesh(np.asarray(devices), ("core",))
    nmaps = n_params + len(out_names)
    sharded = jax.jit(
        shard_map(
            _body,
            mesh=mesh,
            in_specs=(PartitionSpec("core"),) * nmaps,
            out_specs=(PartitionSpec("core"),) * len(out_names),
            check_rep=False,
        ),
        keep_unused=True,
    )
    sh = NamedSharding(mesh, PartitionSpec("core"))
    staged = [
        jax.device_put(
            np.concatenate([np.asarray(m[name]) for m in in_maps], axis=0), sh
        )
        for name in in_names
    ] + [
        jax.device_put(
            np.zeros((n_cores * z.shape[0], *z.shape[1:]), z.dtype), sh
        )
        for z in zero_outs
    ]

    def run(full=False):
        outs = sharded(*staged)
        if full:
            return [np.asarray(o) for o in outs]
        # under axon, block_until_ready alone doesn't track remote
        # completion reliably -- read back one shard as a completion token
        # (small, so readback noise stays out of the timing)
        return [np.asarray(o.addressable_shards[0].data) for o in outs]

    return run


def kernel(e, dst, n_nodes):
    global LAST_RESULT
    e = np.ascontiguousarray(np.asarray(e), dtype=np.float32)
    dst = np.asarray(dst).astype(np.int64)
    assert int(n_nodes) == N and e.shape == (E, D) and dst.shape == (E,)

    e_jds, cnt_pb, order, Bj, Cj, SumB, maxdeg, B, m = _preprocess(e, dst)

    nc = _build_program(SumB, Bj, Cj, maxdeg, B)
    in_maps = _make_in_maps(e_jds, cnt_pb, SumB)
    res = run_bass_kernel_spmd(
        nc,
        in_maps,
        core_ids=list(range(NCORES)),
        trace=TRACE,
        **TRACE_KWARGS,
    )
    LAST_RESULT = res

    out_full = np.zeros((N, D), np.float32)
    ranks = np.arange(m, dtype=np.int64)
    for c in range(NCORES):
        A = np.asarray(res.results[c]["out"]).astype(np.float32)
        A = A.reshape(P, B, D)
        # rank r lives at [r % P, r // P]; rank r is node order[8r + c]
        vals = A.transpose(1, 0, 2).reshape(B * P, D)[:m]
        out_full[order[c + NCORES * ranks]] = vals
    return out_full


def benchmark(e, dst, n_nodes, r_lo=4, r_hi=24, calls=8, **build_kw):
    """Estimate steady-state per-invocation HW time via the slope method:
    two programs with the kernel body repeated r_lo / r_hi times; the
    difference in min wall time isolates on-device time from RPC/staging
    overhead (inputs are staged on-device once per program).
    Returns (ns_per_invocation, details_dict)."""
    import time

    e = np.ascontiguousarray(np.asarray(e), dtype=np.float32)
    dst = np.asarray(dst).astype(np.int64)
    e_jds, cnt_pb, order, Bj, Cj, SumB, maxdeg, B, m = _preprocess(e, dst)
    in_maps = _make_in_maps(e_jds, cnt_pb, SumB)

    results = {}
    for R in (r_lo, r_hi):
        nc = _build_program(SumB, Bj, Cj, maxdeg, B, loop_repeats=R, **build_kw)
        run = _make_runner(nc, in_maps)
        run()  # compile + warmup
        run()
        times = []
        for _ in range(calls):
            t0 = time.perf_counter()
            run()
            times.append(time.perf_counter() - t0)
        results[R] = times
        print(f"R={R}: times(ms) = {[f'{t*1e3:.2f}' for t in sorted(times)]}")

    tau = (min(results[r_hi]) - min(results[r_lo])) / (r_hi - r_lo)
    return tau * 1e9, results


# revision 18
# speedup vs baseline: 1.2422x; 1.2422x over previous
"""Segment-mean (GNN mean-encoder) Trainium2 kernel.

Strategy (per the node-sharding variant of the sharding hint):
  * Host: partition nodes across the 8 cores round-robin in degree-sorted
    order, and repack the edge features into a jagged-diagonal (JDS) layout:
    slot j holds the j-th edge of every node that has > j edges.  Nodes are
    ranked by in-degree (descending), so slot j covers a contiguous prefix
    of ranks and each per-core slot becomes a dense [128, Bj*D] tile
    (rank r -> partition r%128, block r//128), padded only up to 128-row
    slot boundaries (~1.3% overhead).  The stream is stored float16
    (l2 err ~1e-3 vs the 2e-2 gate): halves HBM traffic (the bottleneck)
    and doubles DVE add throughput (2x perf mode needs 2-byte operands).
    Whole slots are greedily grouped into chunk slabs (one dense HBM
    extent each, one input dram tensor per chunk) so every stream DMA is
    one ~1.5 MB contiguous read and every slot needs exactly one DVE op.
  * Device (one SPMD program on 8 NeuronCores): stream the chunks and
    accumulate even slots into A_e, odd slots into A_o -- consecutive DVE
    ops then touch alternating buffers, so the in-place RAW chain (and its
    per-op SBUF write-ack stall, which dominates the small tail slots) is
    broken.  Slots 0/1 are straight copies (4x mode, no memset).  The
    reciprocal 1/max(count,1) is computed from the host-packed per-rank
    degree vector and pre-broadcast to [128, B*D] on the otherwise-idle
    Activation engine, so the finalize multiplies also run in 2x mode.
    Finished block batches get merge-add (A_e += A_o), multiply, and a
    store on the second HWDGE ring, all overlapped with the stream.
  * Host: inverse-permute the per-core outputs back to node order.

No cross-core communication is needed: each core owns a disjoint node set.
"""

import numpy as np
import ml_dtypes

import concourse.bass as bass
import concourse.tile as tile
from concourse import mybir
from concourse.bass_utils import run_bass_kernel_spmd

P = 128          # SBUF partitions
NCORES = 8
D = 32           # feature dim
N = 100000       # nodes
E = 1600000      # edges
CHUNK_BLOCKS = 200   # target D-element column blocks per streamed DMA slab
STREAM_BUFS = 8      # in-flight stream tiles

# test-harness hooks (the grading harness just calls kernel())
TRACE = False
TRACE_KWARGS = {}
LAST_RESULT = None


def _plan(dst, chunk_blocks=None):
    """Layout plan from dst alone: JDS slot geometry + slot-aligned chunks."""
    chunk_blocks = chunk_blocks or CHUNK_BLOCKS
    counts = np.bincount(dst, minlength=N)
    maxdeg = int(counts.max())
    order = np.argsort(-counts, kind="stable")           # nodes, degree desc
    m = N // NCORES                                      # nodes per core
    B = (m + P - 1) // P                                 # accumulator blocks

    counts_sorted = counts[order]
    L = np.zeros((NCORES, maxdeg), np.int64)             # slot lengths
    for c in range(NCORES):
        cc = counts_sorted[c::NCORES]
        hist = np.bincount(cc, minlength=maxdeg + 1)
        L[c, :] = m - np.cumsum(hist)[:maxdeg]
    Bj = np.max((L + P - 1) // P, axis=0)                # blocks per slot
    Cj = np.concatenate([[0], np.cumsum(Bj)]).astype(np.int64)
    SumB = int(Cj[-1])

    # chunk bounds: greedy whole-slot groups of width <= chunk_blocks
    # (oversized single slots are split; segments handle that generically)
    cb = [0]
    for j in range(maxdeg):
        s1 = int(Cj[j + 1])
        while s1 - cb[-1] > chunk_blocks:
            cb.append(cb[-1] + chunk_blocks)
        # close the chunk at this slot's end if the next slot won't fit
        if (
            j + 1 < maxdeg
            and int(Cj[j + 2]) - cb[-1] > chunk_blocks
            and s1 > cb[-1]
        ):
            cb.append(s1)
    if cb[-1] < SumB:
        cb.append(SumB)

    # trailing run of single-block slots (each adds one D-column into block
    # 0): isolate as the tiny final chunk and reduce them in ONE strided
    # reduce_sum op -- that is the post-last-DMA drain, so keep it short
    j1 = maxdeg
    while j1 > 0 and Bj[j1 - 1] == 1:
        j1 -= 1
    if maxdeg - j1 >= 4:
        cut = int(Cj[j1])
        if cut not in cb:
            cb = sorted(set(cb) | {cut})
    else:
        j1 = None
    return dict(
        counts=counts, order=order, counts_sorted=counts_sorted,
        maxdeg=maxdeg, m=m, B=B, Bj=Bj, Cj=Cj, SumB=SumB, cb=cb, j1=j1,
    )


def _pack(e, dst, plan):
    """Build per-core chunk slabs (f16, one dense extent each) + counts."""
    order, Cj, SumB = plan["order"], plan["Cj"], plan["SumB"]
    m, B, cb = plan["m"], plan["B"], plan["cb"]
    inv = np.empty(N, np.int64)
    inv[order] = np.arange(N)
    core_of = inv % NCORES
    rank_of = inv // NCORES

    # per-edge slot index = occurrence index within its dst group
    perm = np.argsort(dst, kind="stable")
    sd = dst[perm]
    newgrp = np.r_[True, sd[1:] != sd[:-1]]
    starts = np.flatnonzero(newgrp)
    group_id = np.cumsum(newgrp.astype(np.int64)) - 1
    j_e = np.arange(E, dtype=np.int64) - starts[group_id]

    c_e = core_of[sd]
    r_e = rank_of[sd]
    flat_idx = (r_e % P) * SumB + Cj[j_e] + (r_e // P)   # row in [P*SumB, D]

    e_jds = np.zeros((NCORES, P, SumB, D), np.float16)
    ejv = e_jds.reshape(NCORES, P * SumB, D)
    for c in range(NCORES):
        mask = c_e == c
        ejv[c, flat_idx[mask]] = e[perm[mask]].astype(np.float16)

    flat = e_jds.reshape(NCORES, P, SumB * D)
    slabs = [
        np.ascontiguousarray(flat[:, :, b0 * D: b1 * D])
        for b0, b1 in zip(cb[:-1], cb[1:])
    ]

    # per-rank in-degree, packed rank r -> [r % P, r // P]; exact in f16.
    # Ranks >= m (padding) get 0 -> output 0 (DGL zero-fill semantics).
    cnt = np.zeros((NCORES, P * B), np.float16)
    for c in range(NCORES):
        cnt[c, :m] = plan["counts_sorted"][c::NCORES]
    cnt_pb = np.ascontiguousarray(
        cnt.reshape(NCORES, B, P).transpose(0, 2, 1)     # [c, P, B]
    )
    return slabs, cnt_pb


def _split_multi_waits(nc):
    """Walrus in this toolchain rejects instructions with more than one sem
    wait ("Too many sync wait commands").  Tile's wait assignment is not
    transitively minimal, so e.g. a DMA reusing a pool slot waits on both the
    consumer engine's sem and its own lane's previous DMA.  Hoist all but one
    wait of each instruction onto same-engine NoOps inserted right before it:
    the sequencer executes them in order, so semantics are identical.
    """
    ctr = 0
    for fn in nc.m.functions:
        for bb in fn.blocks:
            new_insts = []
            for inst in bb.instructions:
                si = inst.sync_info
                if si is not None and si.on_wait and len(si.on_wait) > 1:
                    waits = list(si.on_wait)
                    for w in waits[:-1]:
                        ctr += 1
                        nop = mybir.InstNoOp(
                            name=f"I-waitsplit-{ctr}",
                            engine=inst.engine,
                            ins=[],
                            outs=[],
                            sync_info=mybir.SyncInfo(on_wait=[w], on_update=[]),
                        )
                        new_insts.append(nop)
                    si.on_wait = [waits[-1]]
                new_insts.append(inst)
            bb.instructions = new_insts


def _build_program(
    plan,
    repeats=1,
    loop_repeats=None,
    stream_bufs=None,
    min_fin_blocks=8,
    store_engine="scalar",
    dma_only=False,
    split_acc=True,
    expand_recip=True,
    alt_rings=False,
):
    stream_bufs = stream_bufs or STREAM_BUFS
    Bj, Cj, cb = plan["Bj"], plan["Cj"], plan["cb"]
    maxdeg, B = plan["maxdeg"], plan["B"]

    nc = bass.Bass()
    f16 = mybir.dt.float16
    widths = [b1 - b0 for b0, b1 in zip(cb[:-1], cb[1:])]
    wmax = max(widths)
    ejds = [
        nc.dram_tensor(f"ejds{t}", [P, w * D], f16, kind="ExternalInput")
        for t, w in enumerate(widths)
    ]
    cnts = nc.dram_tensor("cnts", [P, B], f16, kind="ExternalInput")
    out = nc.dram_tensor("out", [P, B * D], f16, kind="ExternalOutput")

    Bj_l = [int(x) for x in Bj] + [0]
    Cj_l = [int(x) for x in Cj]
    store_eng = getattr(nc, store_engine)
    Bj0 = Bj_l[0]
    Bj1 = Bj_l[1] if maxdeg > 1 else 0

    with tile.TileContext(nc) as tc:
        with (
            tc.tile_pool(name="acc", bufs=1) as acc_pool,
            tc.tile_pool(name="small", bufs=2) as small_pool,
            tc.tile_pool(name="stream", bufs=stream_bufs) as stream_pool,
        ):
            A = acc_pool.tile([P, B * D], f16)
            if split_acc:
                Ao = acc_pool.tile([P, max(Bj1, 1) * D], f16)
            else:
                Ao = None

            def emit_body():
                # ablation mode: stream DMAs only, no compute
                if dma_only:
                    first = None
                    for t, w in enumerate(widths):
                        tl = stream_pool.tile([P, wmax * D], f16, tag="stream")
                        nc.sync.dma_start(tl[:, : w * D], ejds[t][:])
                        if first is None:
                            first = tl
                    # mimic the real kernel's 5-batch store pattern
                    nb = 5
                    for k in range(nb):
                        b0 = k * B // nb
                        b1 = (k + 1) * B // nb
                        store_eng.dma_start(
                            out[:, b0 * D: b1 * D],
                            first[:, b0 * D: b1 * D],
                        )
                    return

                # recip = 1/max(count,1), then broadcast-expanded to
                # [P, B*D] on the idle ACT engine so finalize multiplies
                # run in 2x DVE mode; all in the early-stream idle window
                cnt_sb = small_pool.tile([P, B], f16, tag="cnt_sb")
                store_eng.dma_start(cnt_sb[:], cnts[:])
                recip = small_pool.tile([P, B], f16, tag="recip")
                nc.vector.tensor_scalar_max(recip[:], cnt_sb[:], 1.0)
                with nc.allow_low_precision(
                    reason="f16 mean is well within the 2e-2 error gate"
                ):
                    nc.vector.reciprocal(recip[:], recip[:])
                if expand_recip:
                    recip_big = small_pool.tile([P, B * D], f16, tag="rbig")
                    nc.scalar.copy(
                        recip_big[:].rearrange("p (b d) -> p b d", d=D),
                        recip[:, :, None].broadcast_to([P, B, D]),
                    )

                # even slots accumulate into A, odd slots into Ao; blocks
                # no slot touches must be zero (slots 0/1 are copies)
                if Bj0 < B:
                    nc.vector.memset(A[:, Bj0 * D:], 0.0)

                # finalized := blocks >= fin_lo are merged + mult + stored
                fin_lo = [B]

                def finalize_down_to(b0):
                    b1 = fin_lo[0]
                    if b1 <= b0:
                        return
                    if split_acc and b0 < Bj1:
                        mb1 = min(b1, Bj1)
                        nc.vector.tensor_add(
                            A[:, b0 * D: mb1 * D],
                            A[:, b0 * D: mb1 * D],
                            Ao[:, b0 * D: mb1 * D],
                        )
                    if expand_recip:
                        nc.vector.tensor_mul(
                            A[:, b0 * D: b1 * D],
                            A[:, b0 * D: b1 * D],
                            recip_big[:, b0 * D: b1 * D],
                        )
                    else:
                        nc.vector.tensor_mul(
                            A[:, b0 * D: b1 * D].rearrange(
                                "p (b d) -> p b d", d=D
                            ),
                            A[:, b0 * D: b1 * D].rearrange(
                                "p (b d) -> p b d", d=D
                            ),
                            recip[:, b0:b1, None].broadcast_to(
                                [P, b1 - b0, D]
                            ),
                        )
                    store_eng.dma_start(
                        out[:, b0 * D: b1 * D], A[:, b0 * D: b1 * D]
                    )
                    fin_lo[0] = b0

                # stream the chunk slabs; each slot-aligned segment adds
                # into its parity's accumulator.  When slot j's columns
                # end, blocks [Bj[j+1], Bj[j]) are final (later slots only
                # touch lower blocks): merge+multiply+store them in
                # >= min_fin_blocks batches.
                j1 = plan.get("j1")
                jmax = j1 if j1 is not None else maxdeg
                for t, w in enumerate(widths):
                    blk0, blk1 = cb[t], cb[t + 1]
                    tl = stream_pool.tile([P, wmax * D], f16, tag="stream")
                    ring = nc.scalar if (alt_rings and t % 2) else nc.sync
                    ring.dma_start(tl[:, : w * D], ejds[t][:])
                    if j1 is not None and blk0 == Cj_l[j1]:
                        # trailing single-block slots: one strided reduce
                        # (sum over the slot axis) + one add into block 0
                        ntail = blk1 - blk0
                        red = small_pool.tile([P, D], f16, tag="red")
                        with nc.allow_low_precision(
                            reason="f16 mean is well within the 2e-2 gate"
                        ):
                            nc.vector.reduce_sum(
                                red[:, :, None],
                                tl[:, : ntail * D].rearrange(
                                    "p (k d) -> p d k", d=D
                                ),
                                axis=mybir.AxisListType.X,
                            )
                        nc.vector.tensor_add(A[:, :D], A[:, :D], red[:])
                        finalize_down_to(0)
                        continue
                    j = int(np.searchsorted(Cj, blk0, side="right")) - 1
                    while j < jmax and Cj_l[j] < blk1:
                        s0 = max(blk0, Cj_l[j])
                        s1 = min(blk1, Cj_l[j + 1])
                        if s1 > s0:
                            alo = (s0 - Cj_l[j]) * D
                            ww = (s1 - s0) * D
                            t0 = (s0 - blk0) * D
                            tgt = Ao if (split_acc and j % 2 == 1) else A
                            if j == 0 or (split_acc and j == 1):
                                nc.vector.tensor_copy(
                                    tgt[:, alo: alo + ww],
                                    tl[:, t0: t0 + ww],
                                )
                            else:
                                nc.vector.tensor_add(
                                    tgt[:, alo: alo + ww],
                                    tgt[:, alo: alo + ww],
                                    tl[:, t0: t0 + ww],
                                )
                            if s1 == Cj_l[j + 1] and (
                                fin_lo[0] - Bj_l[j + 1] >= min_fin_blocks
                                or j == maxdeg - 1
                            ):
                                finalize_down_to(Bj_l[j + 1])
                        j += 1
                finalize_down_to(0)

            if loop_repeats is not None:
                with tc.For_i(0, loop_repeats, 1):
                    emit_body()
            else:
                for _rep in range(repeats):
                    emit_body()
    _split_multi_waits(nc)
    return nc


def _make_in_maps(slabs, cnt_pb):
    return [
        {
            **{f"ejds{t}": slabs[t][c] for t in range(len(slabs))},
            "cnts": cnt_pb[c],
        }
        for c in range(NCORES)
    ]


def _make_runner(nc, in_maps):
    """Build a repeat-callable PJRT runner with inputs staged on-device once.

    Mirrors bass2jax.run_bass_via_pjrt's multi-core path, minus output-buffer
    donation (so the staged arrays can be reused across timing calls).
    """
    import jax
    from jax.experimental.shard_map import shard_map
    from jax.sharding import Mesh, NamedSharding, PartitionSpec

    from concourse import bass2jax

    bass2jax.install_neuronx_cc_hook()
    n_cores = len(in_maps)

    partition_name = (
        nc.partition_id_tensor.name if nc.partition_id_tensor else None
    )
    in_names, out_names, out_avals, zero_outs = [], [], [], []
    for alloc in nc.m.functions[0].allocations:
        if not isinstance(alloc, mybir.MemoryLocationSet):
            continue
        name = alloc.memorylocations[0].name
        if alloc.kind == "ExternalInput":
            if name != partition_name:
                in_names.append(name)
        elif alloc.kind == "ExternalOutput":
            out_names.append(name)
            shape = tuple(alloc.tensor_shape)
            dtype = mybir.dt.np(alloc.dtype)
            out_avals.append(jax.core.ShapedArray(shape, dtype))
            zero_outs.append(np.zeros(shape, dtype))
    n_params = len(in_names)
    all_names = in_names + out_names
    if partition_name is not None:
        all_names = all_names + [partition_name]

    def _body(*args):
        operands = list(args)
        if partition_name is not None:
            operands.append(bass2jax.partition_id_tensor())
        outs = bass2jax._bass_exec_p.bind(
            *operands,
            out_avals=tuple(out_avals),
            in_names=tuple(all_names),
            out_names=tuple(out_names),
            lowering_input_output_aliases=(),
            sim_require_finite=True,
            sim_require_nnan=True,
            nc=nc,
        )
        return tuple(outs)

    devices = jax.devices()[:n_cores]
    mesh = Mesh(np.asarray(devices), ("core",))
    nmaps = n_params + len(out_names)
    sharded = jax.jit(
        shard_map(
            _body,
            mesh=mesh,
            in_specs=(PartitionSpec("core"),) * nmaps,
            out_specs=(PartitionSpec("core"),) * len(out_names),
            check_rep=False,
        ),
        keep_unused=True,
    )
    sh = NamedSharding(mesh, PartitionSpec("core"))
    staged = [
        jax.device_put(
            np.concatenate([np.asarray(m[name]) for m in in_maps], axis=0), sh
        )
        for name in in_names
    ] + [
        jax.device_put(
            np.zeros((n_cores * z.shape[0], *z.shape[1:]), z.dtype), sh
        )
        for z in zero_outs
    ]

    def run(full=False):
        outs = sharded(*staged)
        if full:
            return [np.asarray(o) for o in outs]
        # under axon, block_until_ready alone doesn't track remote
        # completion reliably -- read back one shard as a completion token
        # (small, so readback noise stays out of the timing)
        return [np.asarray(o.addressable_shards[0].data) for o in outs]

    return run


def kernel(e, dst, n_nodes):
    global LAST_RESULT
    e = np.ascontiguousarray(np.asarray(e), dtype=np.float32)
    dst = np.asarray(dst).astype(np.int64)
    assert int(n_nodes) == N and e.shape == (E, D) and dst.shape == (E,)

    plan = _plan(dst)
    slabs, cnt_pb = _pack(e, dst, plan)

    nc = _build_program(plan)
    in_maps = _make_in_maps(slabs, cnt_pb)
    res = run_bass_kernel_spmd(
        nc,
        in_maps,
        core_ids=list(range(NCORES)),
        trace=TRACE,
        **TRACE_KWARGS,
    )
    LAST_RESULT = res

    order, m, B = plan["order"], plan["m"], plan["B"]
    out_full = np.zeros((N, D), np.float32)
    ranks = np.arange(m, dtype=np.int64)
    for c in range(NCORES):
        A = np.asarray(res.results[c]["out"]).astype(np.float32)
        A = A.reshape(P, B, D)
        # rank r lives at [r % P, r // P]; rank r is node order[8r + c]
        vals = A.transpose(1, 0, 2).reshape(B * P, D)[:m]
        out_full[order[c + NCORES * ranks]] = vals
    return out_full


def benchmark(e, dst, n_nodes, r_lo=8, r_hi=4008, calls=8, chunk_blocks=None,
              **build_kw):
    """Estimate steady-state per-invocation HW time via the slope method:
    two programs with the kernel body repeated r_lo / r_hi times (hardware
    For_i loop); the difference in wall time isolates on-device time from
    RPC/staging overhead (inputs are staged on-device once per program).
    Calls to the two programs are INTERLEAVED so the RPC base drift (the
    axon tunnel's wall overhead is bimodal at the ~10 ms level) cancels in
    adjacent-pair differences; r_hi is large enough that the slope term
    (~160 ms) dwarfs the residual drift.
    Returns (ns_per_invocation, details_dict)."""
    import time

    e = np.ascontiguousarray(np.asarray(e), dtype=np.float32)
    dst = np.asarray(dst).astype(np.int64)
    plan = _plan(dst, chunk_blocks=chunk_blocks)
    slabs, cnt_pb = _pack(e, dst, plan)
    in_maps = _make_in_maps(slabs, cnt_pb)

    runners = {}
    for R in (r_lo, r_hi):
        nc = _build_program(plan, loop_repeats=R, **build_kw)
        runners[R] = _make_runner(nc, in_maps)
        runners[R]()  # compile + warmup
    results = {r_lo: [], r_hi: []}
    for _ in range(calls):
        for R in (r_lo, r_hi):
            t0 = time.perf_counter()
            runners[R]()
            results[R].append(time.perf_counter() - t0)
    for R in (r_lo, r_hi):
        print(f"R={R}: times(ms) = {[f'{t*1e3:.2f}' for t in sorted(results[R])]}")
    diffs = sorted(
        (hi - lo) / (r_hi - r_lo)
        for lo, hi in zip(results[r_lo], results[r_hi])
    )
    tau = diffs[len(diffs) // 2]  # median of per-pair slopes
    print(f"slopes(us) = {[f'{d*1e6:.1f}' for d in diffs]}")
    return tau * 1e9, results
